# revision 2
# baseline (speedup 1.0000x reference)
"""Trainium2 Bass kernel for nn_Hierarch_RNN (hierarchical 2-layer GRU).

Strategy: data-parallel over the batch dim (32 batches -> 4 per core, 8 cores).
On-chip layout is feature-major [d, rows]; rows per core = 4*321 = 1284,
processed in 3 chunks of 428 columns (PSUM bank = 512 fp32 max).

Per GRU step (both layers, encoder + decoder reuse the same emitter):
  - x-side and h-side matmuls accumulate into one PSUM group for the r/z
    gates (sigmoid applied straight from PSUM with the bias via ScalarE).
  - n gate keeps x/h sides in separate PSUM tiles; fused DVE
    scalar_tensor_tensor computes (h_n + bhh_n) * r; tanh adds bih_n.
  - h' = n + z*(h - n) in three DVE tensor-tensor ops per block-chunk.
All matmuls run in float32r (full-rate fp32 mode, ~1e-4 rel err).
"""
import numpy as np

import concourse.mybir as mybir
import concourse.tile as tile
from concourse import bacc
from concourse.bass_utils import run_bass_kernel_spmd

F32 = mybir.dt.float32
F32R = mybir.dt.float32r
AF = mybir.ActivationFunctionType
ALU = mybir.AluOpType

B, SEQ, PRED, ENC = 32, 720, 96, 321
NCORE, BPC = 8, 4
R = BPC * ENC                 # 1284 rows per core
CH, NCH = 428, 3              # row chunks
# layer params: d, seg_len, n 128-blocks of d (DK == NG), decoder steps S
D0, SG0, DK0, S0, T0 = 512, 48, 4, 2, 15
D1, SG1, DK1, S1, T1 = 256, 24, 2, 4, 60

_CACHE = {}


def _build_nc(l0_steps=T0, l1_steps=T1):
    nc = bacc.Bacc("TRN2", target_bir_lowering=False, debug=False,
                   num_devices=NCORE)

    # ---------------- DRAM tensors ----------------
    xseg0_d = nc.dram_tensor("xseg0", [T0, SG0, R], F32R, kind="ExternalInput")
    xseg1_d = nc.dram_tensor("xseg1", [SG1, 4 * R], F32R, kind="ExternalInput")
    wih_d = [nc.dram_tensor("wihT0", [DK0, 128, 3 * D0], F32R, kind="ExternalInput"),
             nc.dram_tensor("wihT1", [DK1, 128, 3 * D1], F32R, kind="ExternalInput")]
    whh_d = [nc.dram_tensor("whhT0", [DK0, 128, 3 * D0], F32R, kind="ExternalInput"),
             nc.dram_tensor("whhT1", [DK1, 128, 3 * D1], F32R, kind="ExternalInput")]
    wemb_d = [nc.dram_tensor("wembT0", [SG0, D0], F32R, kind="ExternalInput"),
              nc.dram_tensor("wembT1", [SG1, D1], F32R, kind="ExternalInput")]
    wpred_d = [nc.dram_tensor("wpredT0", [DK0, 128, SG0], F32R, kind="ExternalInput"),
               nc.dram_tensor("wpredT1", [DK1, 128, SG1], F32R, kind="ExternalInput")]
    brz_d = [nc.dram_tensor("brz0", [128, 2 * DK0], F32, kind="ExternalInput"),
             nc.dram_tensor("brz1", [128, 2 * DK1], F32, kind="ExternalInput")]
    bihn_d = [nc.dram_tensor("bihn0", [128, DK0], F32, kind="ExternalInput"),
              nc.dram_tensor("bihn1", [128, DK1], F32, kind="ExternalInput")]
    bhhn_d = [nc.dram_tensor("bhhn0", [128, DK0], F32, kind="ExternalInput"),
              nc.dram_tensor("bhhn1", [128, DK1], F32, kind="ExternalInput")]
    bemb_d = [nc.dram_tensor("bemb0", [128, DK0], F32, kind="ExternalInput"),
              nc.dram_tensor("bemb1", [128, DK1], F32, kind="ExternalInput")]
    bpred_d = [nc.dram_tensor("bpred0", [128, 1], F32, kind="ExternalInput"),
               nc.dram_tensor("bpred1", [128, 1], F32, kind="ExternalInput")]
    posx_d = [nc.dram_tensor("posx0", [S0, DK0, 128, R], F32R, kind="ExternalInput"),
              nc.dram_tensor("posx1", [S1, DK1, 128, R], F32R, kind="ExternalInput")]
    y_d = [nc.dram_tensor("y0", [S0, SG0, R], F32, kind="ExternalOutput"),
           nc.dram_tensor("y1", [S1, SG1, R], F32, kind="ExternalOutput")]

    with tile.TileContext(nc) as tc:
        with tc.tile_pool(name="const", bufs=1) as cp, \
             tc.tile_pool(name="x0p", bufs=1) as x0p, \
             tc.tile_pool(name="xep", bufs=6) as xep, \
             tc.tile_pool(name="h0p", bufs=8) as h0p, \
             tc.tile_pool(name="h1p", bufs=4) as h1p, \
             tc.tile_pool(name="posp", bufs=5) as posp, \
             tc.tile_pool(name="hyp", bufs=6) as hyp, \
             tc.tile_pool(name="rp", bufs=2) as rp, \
             tc.tile_pool(name="zp", bufs=2) as zp, \
             tc.tile_pool(name="np_", bufs=2) as np_p, \
             tc.tile_pool(name="sp", bufs=2) as sp, \
             tc.tile_pool(name="tp", bufs=2) as tp, \
             tc.tile_pool(name="up", bufs=2) as up, \
             tc.tile_pool(name="vp", bufs=2) as vp, \
             tc.tile_pool(name="yp", bufs=2) as yp, \
             tc.tile_pool(name="psg", bufs=6, space="PSUM") as psg, \
             tc.tile_pool(name="psy", bufs=2, space="PSUM") as psy:

            # ---------------- load constants ----------------
            def load_w(dram, k_tiles, cols):
                t = cp.tile([128, k_tiles * cols], F32R, tag=f"c_{dram.name}",
                            name=f"c_{dram.name}")
                for k in range(k_tiles):
                    nc.sync.dma_start(t[:, k * cols:(k + 1) * cols], dram[k])
                return t

            wih_sb = [load_w(wih_d[0], DK0, 3 * D0), load_w(wih_d[1], DK1, 3 * D1)]
            whh_sb = [load_w(whh_d[0], DK0, 3 * D0), load_w(whh_d[1], DK1, 3 * D1)]
            wpred_sb = [load_w(wpred_d[0], DK0, SG0), load_w(wpred_d[1], DK1, SG1)]
            wemb_sb = []
            for li, (sg, d) in enumerate(((SG0, D0), (SG1, D1))):
                t = cp.tile([sg, d], F32R, tag=f"c_wemb{li}", name=f"c_wemb{li}")
                nc.sync.dma_start(t[:], wemb_d[li][:])
                wemb_sb.append(t)
            def load_b(dram, cols):
                t = cp.tile([128, cols], F32, tag=f"c_{dram.name}",
                            name=f"c_{dram.name}")
                nc.sync.dma_start(t[:], dram[:])
                return t
            brz_sb = [load_b(brz_d[0], 2 * DK0), load_b(brz_d[1], 2 * DK1)]
            bihn_sb = [load_b(bihn_d[0], DK0), load_b(bihn_d[1], DK1)]
            bhhn_sb = [load_b(bhhn_d[0], DK0), load_b(bhhn_d[1], DK1)]
            bemb_sb = [load_b(bemb_d[0], DK0), load_b(bemb_d[1], DK1)]
            bpred_sb = [load_b(bpred_d[0], 1), load_b(bpred_d[1], 1)]
            xs1 = cp.tile([SG1, 4 * R], F32R, tag="c_xs1", name="c_xs1")
            nc.sync.dma_start(xs1[:], xseg1_d[:])

            LP = [dict(D=D0, DK=DK0, SG=SG0, wih=wih_sb[0], whh=whh_sb[0],
                       wemb=wemb_sb[0], wpred=wpred_sb[0], brz=brz_sb[0],
                       bihn=bihn_sb[0], bhhn=bhhn_sb[0], bemb=bemb_sb[0],
                       bpred=bpred_sb[0]),
                  dict(D=D1, DK=DK1, SG=SG1, wih=wih_sb[1], whh=whh_sb[1],
                       wemb=wemb_sb[1], wpred=wpred_sb[1], brz=brz_sb[1],
                       bihn=bihn_sb[1], bhhn=bhhn_sb[1], bemb=bemb_sb[1],
                       bpred=bpred_sb[1])]

            def wcol(P, wt, k, m):
                """AP of [128,128] weight block: k-tile k, m-tile m of 3d."""
                c0 = k * 3 * P["D"] + m * 128
                return wt[:, c0:c0 + 128]

            def make_xe_embed(li, xsrc_fn):
                """Returns make_xe(c): emits per-chunk embed, returns DK APs."""
                P = LP[li]
                def make_xe(c):
                    aps = []
                    for k in range(P["DK"]):
                        ps = psg.tile([128, CH], F32, tag="ps", name="ps_e")
                        nc.tensor.matmul(ps[:], P["wemb"][:, k * 128:(k + 1) * 128],
                                         xsrc_fn(c), start=True, stop=True)
                        xe = xep.tile([128, CH], F32R, tag="xe", name=f"xe{li}_{k}")
                        nc.scalar.activation(xe[:], ps[:], AF.Relu,
                                             bias=P["bemb"][:, k:k + 1])
                        aps.append(xe[:])
                    return aps
                return make_xe

            def emit_gru(li, make_xe, h_in, hout_ap, first):
                """One fused GRU application over all chunks/blocks.
                make_xe(c) -> list of DK x-side rhs APs [128, CH]
                h_in: list of DK [128, R] tiles (prev h), or None if first
                hout_ap(i, c): output AP [128, CH] (f32r tile slice)
                """
                P = LP[li]
                DK = P["DK"]
                for c in range(NCH):
                    cc = slice(c * CH, (c + 1) * CH)
                    xe = make_xe(c)
                    for i in range(DK):
                        # --- r gate (m = i) ---
                        ps_r = psg.tile([128, CH], F32, tag="ps", name="ps_r")
                        for k in range(DK):
                            nc.tensor.matmul(ps_r[:], wcol(P, P["wih"], k, i),
                                             xe[k], start=(k == 0),
                                             stop=(k == DK - 1 and first))
                        if not first:
                            for k in range(DK):
                                nc.tensor.matmul(ps_r[:], wcol(P, P["whh"], k, i),
                                                 h_in[k][:, cc], start=False,
                                                 stop=(k == DK - 1))
                        r = rp.tile([128, CH], F32)
                        nc.scalar.activation(r[:], ps_r[:], AF.Sigmoid,
                                             bias=P["brz"][:, i:i + 1])
                        # --- z gate (m = DK + i) ---
                        ps_z = psg.tile([128, CH], F32, tag="ps", name="ps_z")
                        for k in range(DK):
                            nc.tensor.matmul(ps_z[:], wcol(P, P["wih"], k, DK + i),
                                             xe[k], start=(k == 0),
                                             stop=(k == DK - 1 and first))
                        if not first:
                            for k in range(DK):
                                nc.tensor.matmul(ps_z[:], wcol(P, P["whh"], k, DK + i),
                                                 h_in[k][:, cc], start=False,
                                                 stop=(k == DK - 1))
                        z = zp.tile([128, CH], F32)
                        nc.scalar.activation(z[:], ps_z[:], AF.Sigmoid,
                                             bias=P["brz"][:, DK + i:DK + i + 1])
                        # --- n gate (m = 2*DK + i) ---
                        ps_in = psg.tile([128, CH], F32, tag="ps", name="ps_in")
                        for k in range(DK):
                            nc.tensor.matmul(ps_in[:], wcol(P, P["wih"], k, 2 * DK + i),
                                             xe[k], start=(k == 0),
                                             stop=(k == DK - 1))
                        t_ = tp.tile([128, CH], F32)
                        if first:
                            nc.vector.tensor_scalar_mul(t_[:], r[:],
                                                        P["bhhn"][:, i:i + 1])
                        else:
                            ps_hn = psg.tile([128, CH], F32, tag="ps", name="ps_hn")
                            for k in range(DK):
                                nc.tensor.matmul(ps_hn[:], wcol(P, P["whh"], k, 2 * DK + i),
                                                 h_in[k][:, cc], start=(k == 0),
                                                 stop=(k == DK - 1))
                            nc.vector.scalar_tensor_tensor(
                                t_[:], ps_hn[:], P["bhhn"][:, i:i + 1], r[:],
                                op0=ALU.add, op1=ALU.mult)
                        s_ = sp.tile([128, CH], F32)
                        nc.vector.tensor_add(s_[:], t_[:], ps_in[:])
                        n = np_p.tile([128, CH], F32)
                        nc.scalar.activation(n[:], s_[:], AF.Tanh,
                                             bias=P["bihn"][:, i:i + 1])
                        # --- h' = n + z*(h-n)  (h=0 when first) ---
                        if first:
                            v = vp.tile([128, CH], F32)
                            nc.vector.tensor_mul(v[:], n[:], z[:])
                            nc.vector.tensor_sub(hout_ap(i, c), n[:], v[:])
                        else:
                            u = up.tile([128, CH], F32)
                            nc.vector.tensor_sub(u[:], h_in[i][:, cc], n[:])
                            v = vp.tile([128, CH], F32)
                            nc.vector.tensor_mul(v[:], u[:], z[:])
                            nc.vector.tensor_add(hout_ap(i, c), n[:], v[:])

            def emit_enc_step(li, t, make_xe, h_in):
                P = LP[li]
                h_pool = h0p if li == 0 else h1p
                h_out = [h_pool.tile([128, R], F32R, tag=f"h{li}", name=f"h{li}_{t}_{k}")
                         for k in range(P["DK"])]
                emit_gru(li, make_xe,
                         h_in, lambda i, c: h_out[i][:, c * CH:(c + 1) * CH],
                         first=(t == 0))
                return h_out

            def emit_decoder(li, s_, h_fin):
                P = LP[li]
                DK, SG = P["DK"], P["SG"]
                hy = {}
                def hout(i, c):
                    t = hyp.tile([128, CH], F32R, tag="hy", name=f"hy{li}_{s_}_{i}_{c}")
                    hy[(i, c)] = t
                    return t[:]
                def make_xe(c):
                    aps = []
                    for k in range(DK):
                        pt = posp.tile([128, CH], F32R, tag="pos", name=f"pos{li}_{s_}_{k}_{c}")
                        nc.sync.dma_start(pt[:], posx_d[li][s_, k, :, c * CH:(c + 1) * CH])
                        aps.append(pt[:])
                    return aps
                emit_gru(li, make_xe, h_fin, hout, first=False)
                for c in range(NCH):
                    cc = slice(c * CH, (c + 1) * CH)
                    ps = psy.tile([SG, CH], F32, tag="psy", name="ps_y")
                    for k in range(DK):
                        nc.tensor.matmul(ps[:], P["wpred"][:, k * SG:(k + 1) * SG],
                                         hy[(k, c)][:], start=(k == 0),
                                         stop=(k == DK - 1))
                    y = yp.tile([SG, CH], F32)
                    nc.scalar.activation(y[:], ps[:], AF.Identity,
                                         bias=P["bpred"][0:SG, 0:1])
                    nc.sync.dma_start(y_d[li][s_, :, cc], y[:])

            # ---------------- encoder ----------------
            h0 = None
            h1 = None
            t1 = 0
            for t in range(l0_steps):
                xs_t = x0p.tile([SG0, R], F32R)
                nc.sync.dma_start(xs_t[:], xseg0_d[t])
                h0 = emit_enc_step(
                    0, t, make_xe_embed(0, lambda c, xs_t=xs_t: xs_t[:, c * CH:(c + 1) * CH]),
                    h0)
                for _ in range(4):
                    if t1 < l1_steps:
                        j = t1 % 4
                        h1 = emit_enc_step(
                            1, t1,
                            make_xe_embed(1, lambda c, j=j: xs1[:, j * R + c * CH:j * R + (c + 1) * CH]),
                            h1)
                        t1 += 1
            while t1 < l1_steps:
                j = t1 % 4
                h1 = emit_enc_step(
                    1, t1,
                    make_xe_embed(1, lambda c, j=j: xs1[:, j * R + c * CH:j * R + (c + 1) * CH]),
                    h1)
                t1 += 1

            # ---------------- decoders ----------------
            emit_decoder(0, 0, h0)
            emit_decoder(1, 0, h1)
            emit_decoder(0, 1, h0)
            emit_decoder(1, 1, h1)
            emit_decoder(1, 2, h1)
            emit_decoder(1, 3, h1)

    nc.compile()
    return nc


def get_nc(l0_steps=T0, l1_steps=T1):
    key = (l0_steps, l1_steps)
    if key not in _CACHE:
        _CACHE[key] = _build_nc(l0_steps, l1_steps)
    return _CACHE[key]


# ==================== host side ====================

def _prep_shared(inp):
    f = np.float32
    m = {}
    for li, d in ((0, D0), (1, D1)):
        dk = (DK0, DK1)[li]
        sg = (SG0, SG1)[li]
        m[f"wembT{li}"] = np.ascontiguousarray(inp[f"W_emb{li}"].T, f)
        m[f"wihT{li}"] = np.ascontiguousarray(
            inp[f"Wih{li}"].T.reshape(dk, 128, 3 * d), f)
        m[f"whhT{li}"] = np.ascontiguousarray(
            inp[f"Whh{li}"].T.reshape(dk, 128, 3 * d), f)
        m[f"wpredT{li}"] = np.ascontiguousarray(
            inp[f"Wpred{li}"].T.reshape(dk, 128, sg), f)
        bih, bhh = inp[f"bih{li}"].astype(f), inp[f"bhh{li}"].astype(f)
        m[f"brz{li}"] = np.ascontiguousarray(
            (bih + bhh)[:2 * d].reshape(2 * dk, 128).T)
        m[f"bihn{li}"] = np.ascontiguousarray(bih[2 * d:].reshape(dk, 128).T)
        m[f"bhhn{li}"] = np.ascontiguousarray(bhh[2 * d:].reshape(dk, 128).T)
        m[f"bemb{li}"] = np.ascontiguousarray(
            inp[f"b_emb{li}"].astype(f).reshape(dk, 128).T)
        bp = np.zeros((128, 1), f)
        bp[:sg, 0] = inp[f"bpred{li}"].astype(f)
        m[f"bpred{li}"] = bp
        half = d // 2
        pos, chan = inp[f"pos{li}"].astype(f), inp[f"chan{li}"].astype(f)
        S = pos.shape[0]
        base = np.concatenate(
            [np.broadcast_to(pos[:, None, :], (S, ENC, half)),
             np.broadcast_to(chan[None, :, :], (S, ENC, half))], axis=-1)
        posx = np.tile(base.transpose(0, 2, 1), (1, 1, BPC))   # [S, d, R]
        m[f"posx{li}"] = np.ascontiguousarray(posx.reshape(S, dk, 128, R))
    return m


def _prep_core(x, c):
    f = np.float32
    xb = x[BPC * c:BPC * (c + 1)].astype(f)
    last = xb[:, -1:, :]
    xc = (xb - last).transpose(0, 2, 1).reshape(R, SEQ)
    xseg0 = np.ascontiguousarray(xc.reshape(R, T0, SG0).transpose(1, 2, 0))
    xseg1 = np.ascontiguousarray(
        xc[:, :4 * SG1].reshape(R, 4, SG1).transpose(2, 1, 0).reshape(SG1, 4 * R))
    return xseg0, xseg1


def make_in_maps(inputs):
    x = np.asarray(inputs["x"], np.float32)
    shared = _prep_shared({k: np.asarray(v) for k, v in inputs.items()})
    in_maps = []
    for c in range(NCORE):
        xseg0, xseg1 = _prep_core(x, c)
        in_maps.append({"xseg0": xseg0, "xseg1": xseg1, **shared})
    return in_maps


def kernel(**inputs):
    in_maps = make_in_maps(inputs)
    nc = get_nc()
    res = run_bass_kernel_spmd(nc, in_maps, list(range(NCORE))).results
    full0 = np.concatenate([res[c]["y0"] for c in range(NCORE)], axis=2)
    full1 = np.concatenate([res[c]["y1"] for c in range(NCORE)], axis=2)
    # out[b, s_*seg+j, e] = y[s_, j, n=(b,e)]
    yl0 = full0.reshape(S0, SG0, B, ENC).transpose(2, 0, 1, 3).reshape(B, PRED, ENC)
    yl1 = full1.reshape(S1, SG1, B, ENC).transpose(2, 0, 1, 3).reshape(B, PRED, ENC)
    return ((yl0 + yl1) / 2.0 + x[:, -1:, :]).astype(np.float32)



# revision 3
# speedup vs baseline: 4.2897x; 4.2897x over previous
"""Trainium2 Bass kernel for nn_Hierarch_RNN (hierarchical 2-layer GRU), v3.

vs v2: all gate matmuls (x-side and h-side, encoder + decoder) run in
fp8e4m3 with perf_mode=DoubleRow, pairing two 128-row contraction
sub-tiles per matmul (2x PE throughput). Mirror-predicted rel err 2.8e-3
(tolerance 2e-2). h state is kept in bf16 for the GRU combine; a Pool
(GpSimd) copy maintains an fp8 shadow [128, DK, 1296] for the next step's
h-side matmuls. gi1 r/z tensors are fp8 (injected via fp8 identity
matmul); gi1 n stays bf16 for the DVE add. Value embeddings stay bf16
(K=48, no DoubleRow possible); their relu output is written as fp8.
"""
import numpy as np
import ml_dtypes

import concourse.mybir as mybir
import concourse.tile as tile
from concourse import bacc
from concourse.bass_utils import run_bass_kernel_spmd

F32 = mybir.dt.float32
BF16 = mybir.dt.bfloat16
FP8 = mybir.dt.float8e4
DR = mybir.MatmulPerfMode.DoubleRow
AF = mybir.ActivationFunctionType
ALU = mybir.AluOpType
BF = ml_dtypes.bfloat16
F8 = ml_dtypes.float8_e4m3

B, SEQ, PRED, ENC = 32, 720, 96, 321
NCORE, BPC = 8, 4
R = BPC * ENC                 # 1284 rows per core
CH, NCH = 428, 3              # row chunks
CHP = 432                     # padded chunk stride (16B aligned for fp8 APs)
RP = NCH * CHP                # 1296
D0, SG0, DK0, S0, T0 = 512, 48, 4, 2, 15
D1, SG1, DK1, S1, T1 = 256, 24, 2, 4, 60

_CACHE = {}


def _build_nc(l0_steps=T0, l1_steps=T1, reps=1):
    nc = bacc.Bacc("TRN2", target_bir_lowering=False, debug=False,
                   num_devices=NCORE)

    xseg0_d = nc.dram_tensor("xseg0", [T0, SG0, R], BF16, kind="ExternalInput")
    xseg1_d = nc.dram_tensor("xseg1", [SG1, 4 * R], BF16, kind="ExternalInput")
    ident_d = nc.dram_tensor("ident", [128, 128], FP8, kind="ExternalInput")
    wih_d = [nc.dram_tensor("wihT0", [DK0, 128, 3 * D0], FP8, kind="ExternalInput"),
             nc.dram_tensor("wihT1", [DK1, 128, 3 * D1], FP8, kind="ExternalInput")]
    whh_d = [nc.dram_tensor("whhT0", [DK0, 128, 3 * D0], FP8, kind="ExternalInput"),
             nc.dram_tensor("whhT1", [DK1, 128, 3 * D1], FP8, kind="ExternalInput")]
    wemb_d = [nc.dram_tensor("wembT0", [SG0, D0], BF16, kind="ExternalInput"),
              nc.dram_tensor("wembT1", [SG1, D1], BF16, kind="ExternalInput")]
    wpred_d = [nc.dram_tensor("wpredT0", [DK0, 128, SG0], BF16, kind="ExternalInput"),
               nc.dram_tensor("wpredT1", [DK1, 128, SG1], BF16, kind="ExternalInput")]
    brz_d = [nc.dram_tensor("brz0", [128, 2 * DK0], F32, kind="ExternalInput"),
             nc.dram_tensor("brz1", [128, 2 * DK1], F32, kind="ExternalInput")]
    bihn_d = [nc.dram_tensor("bihn0", [128, DK0], F32, kind="ExternalInput"),
              nc.dram_tensor("bihn1", [128, DK1], F32, kind="ExternalInput")]
    bhhn_d = [nc.dram_tensor("bhhn0", [128, DK0], F32, kind="ExternalInput"),
              nc.dram_tensor("bhhn1", [128, DK1], F32, kind="ExternalInput")]
    bemb_d = [nc.dram_tensor("bemb0", [128, DK0], F32, kind="ExternalInput"),
              nc.dram_tensor("bemb1", [128, DK1], F32, kind="ExternalInput")]
    bpred_d = [nc.dram_tensor("bpred0", [128, 1], F32, kind="ExternalInput"),
               nc.dram_tensor("bpred1", [128, 1], F32, kind="ExternalInput")]
    posx_d = [nc.dram_tensor("posx0", [S0, DK0, 128, R], FP8, kind="ExternalInput"),
              nc.dram_tensor("posx1", [S1, DK1, 128, R], FP8, kind="ExternalInput")]
    y_d = [nc.dram_tensor("y0", [S0, SG0, R], F32, kind="ExternalOutput"),
           nc.dram_tensor("y1", [S1, SG1, R], F32, kind="ExternalOutput")]

    from contextlib import ExitStack
    with tile.TileContext(nc) as tc:
        with ExitStack() as _stk:
            def mkpool(name, bufs, space="SBUF"):
                return _stk.enter_context(
                    tc.tile_pool(name=name, bufs=bufs, space=space))
            cp = mkpool("const", 1)
            x0p = mkpool("x0p", 2)
            x1p = mkpool("x1p", 4)
            xep = mkpool("xep", 2)      # [128, DK0, RP] fp8 per l0 step
            x1e = mkpool("x1e", 2)      # [128, DK1, RP] fp8 per j
            h8p = mkpool("h8p", 2)      # fp8 shadow of h0
            h18p = mkpool("h18p", 2)    # fp8 shadow of h1
            h0p = mkpool("h0p", 8)
            h1p = mkpool("h1p", 4)
            posp = mkpool("posp", 4)    # [128, DK, CHP] fp8 per (s_, c)
            hyp = mkpool("hyp", 10)
            rp = mkpool("rp", 4)
            zp = mkpool("zp", 4)
            np_p = mkpool("np_", 4)
            sp = mkpool("sp", 4)
            tp = mkpool("tp", 4)
            up = mkpool("up", 3)
            vp = mkpool("vp", 3)
            yp = mkpool("yp", 2)
            psg = mkpool("psg", 6, space="PSUM")
            psy = mkpool("psy", 2, space="PSUM")

            # ---------------- load constants ----------------
            def load_w3(dram, k_tiles, cols, dt):
                t = cp.tile([128, k_tiles, cols], dt, tag=f"c_{dram.name}",
                            name=f"c_{dram.name}")
                for k in range(k_tiles):
                    nc.sync.dma_start(t[:, k, :], dram[k])
                return t

            wih_sb = [load_w3(wih_d[0], DK0, 3 * D0, FP8),
                      load_w3(wih_d[1], DK1, 3 * D1, FP8)]
            whh_sb = [load_w3(whh_d[0], DK0, 3 * D0, FP8),
                      load_w3(whh_d[1], DK1, 3 * D1, FP8)]

            def load_w2(dram, k_tiles, cols):
                t = cp.tile([128, k_tiles * cols], BF16, tag=f"c_{dram.name}",
                            name=f"c_{dram.name}")
                for k in range(k_tiles):
                    nc.sync.dma_start(t[:, k * cols:(k + 1) * cols], dram[k])
                return t
            wpred_sb = [load_w2(wpred_d[0], DK0, SG0), load_w2(wpred_d[1], DK1, SG1)]
            wemb_sb = []
            for li, (sg, d) in enumerate(((SG0, D0), (SG1, D1))):
                t = cp.tile([sg, d], BF16, tag=f"c_wemb{li}", name=f"c_wemb{li}")
                nc.sync.dma_start(t[:], wemb_d[li][:])
                wemb_sb.append(t)
            ident = cp.tile([128, 128], FP8, tag="c_ident", name="c_ident")
            nc.sync.dma_start(ident[:], ident_d[:])

            def load_b(dram, cols):
                t = cp.tile([128, cols], F32, tag=f"c_{dram.name}",
                            name=f"c_{dram.name}")
                nc.sync.dma_start(t[:], dram[:])
                return t
            brz_sb = [load_b(brz_d[0], 2 * DK0), load_b(brz_d[1], 2 * DK1)]
            bihn_sb = [load_b(bihn_d[0], DK0), load_b(bihn_d[1], DK1)]
            bhhn_sb = [load_b(bhhn_d[0], DK0), load_b(bhhn_d[1], DK1)]
            bemb_sb = [load_b(bemb_d[0], DK0), load_b(bemb_d[1], DK1)]
            bpred_sb = [load_b(bpred_d[0], 1), load_b(bpred_d[1], 1)]

            LP = [dict(D=D0, DK=DK0, SG=SG0, wih=wih_sb[0], whh=whh_sb[0],
                       wemb=wemb_sb[0], wpred=wpred_sb[0], brz=brz_sb[0],
                       bihn=bihn_sb[0], bhhn=bhhn_sb[0], bemb=bemb_sb[0],
                       bpred=bpred_sb[0]),
                  dict(D=D1, DK=DK1, SG=SG1, wih=wih_sb[1], whh=whh_sb[1],
                       wemb=wemb_sb[1], wpred=wpred_sb[1], brz=brz_sb[1],
                       bihn=bihn_sb[1], bhhn=bhhn_sb[1], bemb=bemb_sb[1],
                       bpred=bpred_sb[1])]

            def cc_of(c):
                return slice(c * CH, (c + 1) * CH)

            def pc_of(c):
                return slice(c * CHP, c * CHP + CH)

            def mm_dr(ps, P, wt, m, rhs3, c, start, stop):
                """Accumulate full d-contraction of gate-col m via DoubleRow
                pairs. rhs3: [128, DK, RP] fp8 tile."""
                DK = P["DK"]
                npair = DK // 2
                for p in range(npair):
                    nc.tensor.matmul(
                        ps[:], wt[:, 2 * p:2 * p + 2, m * 128:(m + 1) * 128],
                        rhs3[:, 2 * p:2 * p + 2, pc_of(c)],
                        start=(start and p == 0), stop=(stop and p == npair - 1),
                        perf_mode=DR)

            def gate_tail(P, i, c, r_, z_, in_src, ps_hn, h_in_ap, h_out_ap,
                          h8_out, first):
                t_ = tp.tile([128, CH], BF16, tag="t", name="t_")
                if first:
                    nc.vector.tensor_scalar_mul(t_[:], r_[:], P["bhhn"][:, i:i + 1])
                else:
                    nc.vector.scalar_tensor_tensor(
                        t_[:], ps_hn[:], P["bhhn"][:, i:i + 1], r_[:],
                        op0=ALU.add, op1=ALU.mult)
                s_ = sp.tile([128, CH], BF16, tag="s", name="s_")
                nc.vector.tensor_add(s_[:], t_[:], in_src)
                n_ = np_p.tile([128, CH], BF16, tag="n", name="n_")
                nc.scalar.activation(n_[:], s_[:], AF.Tanh,
                                     bias=P["bihn"][:, i:i + 1])
                v = vp.tile([128, CH], BF16, tag="v", name="v_")
                if first:
                    nc.vector.tensor_mul(v[:], n_[:], z_[:])
                    nc.vector.tensor_sub(h_out_ap, n_[:], v[:])
                else:
                    u = up.tile([128, CH], BF16, tag="u", name="u_")
                    nc.vector.tensor_sub(u[:], h_in_ap, n_[:])
                    nc.vector.tensor_mul(v[:], u[:], z_[:])
                    nc.vector.tensor_add(h_out_ap, n_[:], v[:])
                if h8_out is not None:
                    nc.gpsimd.tensor_copy(h8_out, h_out_ap)

            def emit_gi1(gi8, gin):
                for j in range(min(4, l1_steps)):
                    xe3 = x1e.tile([128, DK1, RP], FP8, tag="xe1", name=f"xe1_{j}")
                    for c in range(NCH):
                        xs = x1p.tile([SG1, CH], BF16, tag="xs1",
                                      name=f"xs1_{j}_{c}")
                        nc.sync.dma_start(
                            xs[:], xseg1_d[:, j * R + c * CH:j * R + (c + 1) * CH])
                        for k in range(DK1):
                            ps = psg.tile([128, CH], F32, tag="ps", name="ps_e1")
                            nc.tensor.matmul(
                                ps[:], wemb_sb[1][:, k * 128:(k + 1) * 128],
                                xs[:], start=True, stop=True)
                            nc.scalar.activation(xe3[:, k, pc_of(c)], ps[:],
                                                 AF.Relu,
                                                 bias=bemb_sb[1][:, k:k + 1])
                        for g in range(3):
                            for i in range(DK1):
                                ps = psg.tile([128, CH], F32, tag="ps", name="ps_g1")
                                mm_dr(ps, LP[1], wih_sb[1], g * DK1 + i, xe3, c,
                                      True, True)
                                dst = (gin[(j, i)][:, cc_of(c)] if g == 2
                                       else gi8[(j, g, i)][:, cc_of(c)])
                                nc.scalar.activation(dst, ps[:], AF.Identity)

            def emit_l0_step(t, h_in, h8_in, xs_t):
                P = LP[0]
                first = h_in is None
                h_out = [h0p.tile([128, R], BF16, tag="h0", name=f"h0_{t}_{k}")
                         for k in range(DK0)]
                h8_out = h8p.tile([128, DK0, RP], FP8, tag="h8", name=f"h8_{t}")
                for c in range(NCH):
                    cc = cc_of(c)
                    xe3 = None
                    if c == 0:
                        xe3 = xep.tile([128, DK0, RP], FP8, tag="xe0",
                                       name=f"xe0_{t}")
                        emit_l0_step.xe3 = xe3
                    xe3 = emit_l0_step.xe3
                    for k in range(DK0):
                        ps = psg.tile([128, CH], F32, tag="ps", name="ps_e0")
                        nc.tensor.matmul(ps[:], P["wemb"][:, k * 128:(k + 1) * 128],
                                         xs_t[:, cc], start=True, stop=True)
                        nc.scalar.activation(xe3[:, k, pc_of(c)], ps[:], AF.Relu,
                                             bias=P["bemb"][:, k:k + 1])
                    for i in range(DK0):
                        ps_r = psg.tile([128, CH], F32, tag="ps", name="ps_r")
                        mm_dr(ps_r, P, P["wih"], i, xe3, c, True, first)
                        if not first:
                            mm_dr(ps_r, P, P["whh"], i, h8_in, c, False, True)
                        r_ = rp.tile([128, CH], BF16, tag="r", name="r_")
                        nc.scalar.activation(r_[:], ps_r[:], AF.Sigmoid,
                                             bias=P["brz"][:, i:i + 1])
                        ps_z = psg.tile([128, CH], F32, tag="ps", name="ps_z")
                        mm_dr(ps_z, P, P["wih"], DK0 + i, xe3, c, True, first)
                        if not first:
                            mm_dr(ps_z, P, P["whh"], DK0 + i, h8_in, c, False, True)
                        z_ = zp.tile([128, CH], BF16, tag="z", name="z_")
                        nc.scalar.activation(z_[:], ps_z[:], AF.Sigmoid,
                                             bias=P["brz"][:, DK0 + i:DK0 + i + 1])
                        ps_in = psg.tile([128, CH], F32, tag="ps", name="ps_in")
                        mm_dr(ps_in, P, P["wih"], 2 * DK0 + i, xe3, c, True, True)
                        ps_hn = None
                        if not first:
                            ps_hn = psg.tile([128, CH], F32, tag="ps", name="ps_hn")
                            mm_dr(ps_hn, P, P["whh"], 2 * DK0 + i, h8_in, c,
                                  True, True)
                        gate_tail(P, i, c, r_, z_, ps_in[:], ps_hn,
                                  None if first else h_in[i][:, cc],
                                  h_out[i][:, cc], h8_out[:, i, pc_of(c)], first)
                return h_out, h8_out

            def emit_l1_step(t1, h_in, h8_in, gi8, gin):
                P = LP[1]
                first = h_in is None
                j = t1 % 4
                h_out = [h1p.tile([128, R], BF16, tag="h1", name=f"h1_{t1}_{k}")
                         for k in range(DK1)]
                h8_out = h18p.tile([128, DK1, RP], FP8, tag="h18", name=f"h18_{t1}")
                for c in range(NCH):
                    cc = cc_of(c)
                    for i in range(DK1):
                        ps_r = psg.tile([128, CH], F32, tag="ps", name="ps_r1")
                        nc.tensor.matmul(ps_r[:], ident[:], gi8[(j, 0, i)][:, cc],
                                         start=True, stop=first)
                        if not first:
                            mm_dr(ps_r, P, P["whh"], i, h8_in, c, False, True)
                        r_ = rp.tile([128, CH], BF16, tag="r", name="r1_")
                        nc.scalar.activation(r_[:], ps_r[:], AF.Sigmoid,
                                             bias=P["brz"][:, i:i + 1])
                        ps_z = psg.tile([128, CH], F32, tag="ps", name="ps_z1")
                        nc.tensor.matmul(ps_z[:], ident[:], gi8[(j, 1, i)][:, cc],
                                         start=True, stop=first)
                        if not first:
                            mm_dr(ps_z, P, P["whh"], DK1 + i, h8_in, c, False, True)
                        z_ = zp.tile([128, CH], BF16, tag="z", name="z1_")
                        nc.scalar.activation(z_[:], ps_z[:], AF.Sigmoid,
                                             bias=P["brz"][:, DK1 + i:DK1 + i + 1])
                        ps_hn = None
                        if not first:
                            ps_hn = psg.tile([128, CH], F32, tag="ps", name="ps_hn1")
                            mm_dr(ps_hn, P, P["whh"], 2 * DK1 + i, h8_in, c,
                                  True, True)
                        gate_tail(P, i, c, r_, z_, gin[(j, i)][:, cc], ps_hn,
                                  None if first else h_in[i][:, cc],
                                  h_out[i][:, cc], h8_out[:, i, pc_of(c)], first)
                return h_out, h8_out

            def emit_decoder(li, s_, h_fin, h8_fin):
                P = LP[li]
                DK, SG = P["DK"], P["SG"]
                for c in range(NCH):
                    cc = cc_of(c)
                    pos3 = posp.tile([128, DK, CHP], FP8, tag="pos",
                                     name=f"po{li}_{s_}_{c}")
                    for k in range(DK):
                        nc.sync.dma_start(
                            pos3[:, k, 0:CH],
                            posx_d[li][s_, k, :, c * CH:(c + 1) * CH])
                    hyc = []
                    for i in range(DK):
                        ps_r = psg.tile([128, CH], F32, tag="ps", name="ps_rd")
                        mm_dr(ps_r, P, P["wih"], i, pos3, 0, True, False)
                        mm_dr(ps_r, P, P["whh"], i, h8_fin, c, False, True)
                        r_ = rp.tile([128, CH], BF16, tag="r", name="rd_")
                        nc.scalar.activation(r_[:], ps_r[:], AF.Sigmoid,
                                             bias=P["brz"][:, i:i + 1])
                        ps_z = psg.tile([128, CH], F32, tag="ps", name="ps_zd")
                        mm_dr(ps_z, P, P["wih"], DK + i, pos3, 0, True, False)
                        mm_dr(ps_z, P, P["whh"], DK + i, h8_fin, c, False, True)
                        z_ = zp.tile([128, CH], BF16, tag="z", name="zd_")
                        nc.scalar.activation(z_[:], ps_z[:], AF.Sigmoid,
                                             bias=P["brz"][:, DK + i:DK + i + 1])
                        ps_in = psg.tile([128, CH], F32, tag="ps", name="ps_ind")
                        mm_dr(ps_in, P, P["wih"], 2 * DK + i, pos3, 0, True, True)
                        ps_hn = psg.tile([128, CH], F32, tag="ps", name="ps_hnd")
                        mm_dr(ps_hn, P, P["whh"], 2 * DK + i, h8_fin, c, True, True)
                        t_ = tp.tile([128, CH], BF16, tag="t", name="td")
                        nc.vector.scalar_tensor_tensor(
                            t_[:], ps_hn[:], P["bhhn"][:, i:i + 1], r_[:],
                            op0=ALU.add, op1=ALU.mult)
                        sx = sp.tile([128, CH], BF16, tag="s", name="sd")
                        nc.vector.tensor_add(sx[:], t_[:], ps_in[:])
                        n_ = np_p.tile([128, CH], BF16, tag="n", name="nd")
                        nc.scalar.activation(n_[:], sx[:], AF.Tanh,
                                             bias=P["bihn"][:, i:i + 1])
                        hyt = hyp.tile([128, CH], BF16, tag="hy",
                                       name=f"hy{li}_{s_}_{c}_{i}")
                        u = up.tile([128, CH], BF16, tag="u", name="ud")
                        nc.vector.tensor_sub(u[:], h_fin[i][:, cc], n_[:])
                        v = vp.tile([128, CH], BF16, tag="v", name="vd")
                        nc.vector.tensor_mul(v[:], u[:], z_[:])
                        nc.vector.tensor_add(hyt[:], n_[:], v[:])
                        hyc.append(hyt)
                    ps = psy.tile([SG, CH], F32, tag="psy", name="ps_y")
                    for k in range(DK):
                        nc.tensor.matmul(ps[:], P["wpred"][:, k * SG:(k + 1) * SG],
                                         hyc[k][:], start=(k == 0),
                                         stop=(k == DK - 1))
                    y = yp.tile([SG, CH], F32, tag="y", name="yd")
                    nc.scalar.activation(y[:], ps[:], AF.Identity,
                                         bias=P["bpred"][0:SG, 0:1])
                    nc.sync.dma_start(y_d[li][s_, :, cc], y[:])

            # gi1 const tiles: r/z fp8, n bf16
            gi8 = {}
            gin = {}
            for j in range(min(4, l1_steps)):
                for g in range(2):
                    for i in range(DK1):
                        gi8[(j, g, i)] = cp.tile(
                            [128, R], FP8, tag=f"gi8_{j}_{g}_{i}",
                            name=f"gi8_{j}_{g}_{i}")
                for i in range(DK1):
                    gin[(j, i)] = cp.tile(
                        [128, R], BF16, tag=f"gin_{j}_{i}", name=f"gin_{j}_{i}")

            # ---------------- full iteration (x reps for timing) ----------
            for _rep in range(reps):
                emit_gi1(gi8, gin)
                h0 = h08 = None
                h1 = h18 = None
                t1 = 0
                for t in range(l0_steps):
                    xs_t = x0p.tile([SG0, R], BF16, tag="xs0",
                                    name=f"xs0_{_rep}_{t}")
                    nc.sync.dma_start(xs_t[:], xseg0_d[t])
                    h0, h08 = emit_l0_step(t, h0, h08, xs_t)
                    for _ in range(4):
                        if t1 < l1_steps:
                            h1, h18 = emit_l1_step(t1, h1, h18, gi8, gin)
                            t1 += 1
                while t1 < l1_steps:
                    h1, h18 = emit_l1_step(t1, h1, h18, gi8, gin)
                    t1 += 1

                emit_decoder(0, 0, h0, h08)
                emit_decoder(1, 0, h1, h18)
                emit_decoder(0, 1, h0, h08)
                emit_decoder(1, 1, h1, h18)
                emit_decoder(1, 2, h1, h18)
                emit_decoder(1, 3, h1, h18)

    nc.compile()
    return nc


def get_nc(l0_steps=T0, l1_steps=T1, reps=1):
    key = (l0_steps, l1_steps, reps)
    if key not in _CACHE:
        _CACHE[key] = _build_nc(l0_steps, l1_steps, reps)
    return _CACHE[key]


# ==================== host side ====================

def _prep_shared(inp):
    f = np.float32
    m = {"ident": np.eye(128, dtype=F8)}
    for li, d in ((0, D0), (1, D1)):
        dk = (DK0, DK1)[li]
        sg = (SG0, SG1)[li]
        m[f"wembT{li}"] = np.ascontiguousarray(inp[f"W_emb{li}"].T).astype(BF)
        m[f"wihT{li}"] = np.ascontiguousarray(
            inp[f"Wih{li}"].T.reshape(dk, 128, 3 * d)).astype(F8)
        m[f"whhT{li}"] = np.ascontiguousarray(
            inp[f"Whh{li}"].T.reshape(dk, 128, 3 * d)).astype(F8)
        m[f"wpredT{li}"] = np.ascontiguousarray(
            inp[f"Wpred{li}"].T.reshape(dk, 128, sg)).astype(BF)
        bih, bhh = inp[f"bih{li}"].astype(f), inp[f"bhh{li}"].astype(f)
        m[f"brz{li}"] = np.ascontiguousarray(
            (bih + bhh)[:2 * d].reshape(2 * dk, 128).T)
        m[f"bihn{li}"] = np.ascontiguousarray(bih[2 * d:].reshape(dk, 128).T)
        m[f"bhhn{li}"] = np.ascontiguousarray(bhh[2 * d:].reshape(dk, 128).T)
        m[f"bemb{li}"] = np.ascontiguousarray(
            inp[f"b_emb{li}"].astype(f).reshape(dk, 128).T)
        bp = np.zeros((128, 1), f)
        bp[:sg, 0] = inp[f"bpred{li}"].astype(f)
        m[f"bpred{li}"] = bp
        half = d // 2
        pos, chan = inp[f"pos{li}"].astype(f), inp[f"chan{li}"].astype(f)
        S = pos.shape[0]
        base = np.concatenate(
            [np.broadcast_to(pos[:, None, :], (S, ENC, half)),
             np.broadcast_to(chan[None, :, :], (S, ENC, half))], axis=-1)
        posx = np.tile(base.transpose(0, 2, 1), (1, 1, BPC))   # [S, d, R]
        m[f"posx{li}"] = np.ascontiguousarray(posx.reshape(S, dk, 128, R)).astype(F8)
    return m


def _prep_core(x, c):
    f = np.float32
    xb = x[BPC * c:BPC * (c + 1)].astype(f)
    last = xb[:, -1:, :]
    xc = (xb - last).transpose(0, 2, 1).reshape(R, SEQ)
    xseg0 = np.ascontiguousarray(xc.reshape(R, T0, SG0).transpose(1, 2, 0)).astype(BF)
    xseg1 = np.ascontiguousarray(
        xc[:, :4 * SG1].reshape(R, 4, SG1).transpose(2, 1, 0).reshape(SG1, 4 * R)
    ).astype(BF)
    return xseg0, xseg1


def make_in_maps(inputs):
    x = np.asarray(inputs["x"], np.float32)
    shared = _prep_shared({k: np.asarray(v) for k, v in inputs.items()})
    in_maps = []
    for c in range(NCORE):
        xseg0, xseg1 = _prep_core(x, c)
        in_maps.append({"xseg0": xseg0, "xseg1": xseg1, **shared})
    return in_maps


def kernel(**inputs):
    in_maps = make_in_maps(inputs)
    nc = get_nc()
    res = run_bass_kernel_spmd(nc, in_maps, list(range(NCORE))).results
    full0 = np.concatenate([res[c]["y0"] for c in range(NCORE)], axis=2)
    full1 = np.concatenate([res[c]["y1"] for c in range(NCORE)], axis=2)
    yl0 = full0.reshape(S0, SG0, B, ENC).transpose(2, 0, 1, 3).reshape(B, PRED, ENC)
    yl1 = full1.reshape(S1, SG1, B, ENC).transpose(2, 0, 1, 3).reshape(B, PRED, ENC)
    x = np.asarray(inputs["x"], np.float32)
    return ((yl0 + yl1) / 2.0 + x[:, -1:, :]).astype(np.float32)
